# revision 22
# baseline (speedup 1.0000x reference)
# Laplacian normalization kernel for Trainium2 (8 NeuronCores, SPMD).
#
# out = d^-1/2[:, None] * A * d^-1/2[None, :],  d_i = sum_j A[i, j],  A: [8192, 8192] f32
#
# The rel-err gate is 2e-2; bf16 end-to-end measures ~1.35e-2 max rel
# err on this distribution (A, out, and the gathered scale vector in
# bf16; degree accumulation and d^-1/2 in f32), so the whole data path
# runs in bf16: HBM traffic per core is 32MB (16MB in + 16MB out) vs
# 88MB for the f32 two-pass version, and the full 16MB shard stays
# resident in SBUF (128KB/partition) so nothing is read twice.
#
# Sharding: row-wise across 8 cores (1024 rows each). Row sums are
# local; column scaling needs the full d^-1/2 [8192], gathered in TWO
# bf16 AllGathers (local rows 0-511, then 512-1023) so the second one
# overlaps the scale+store of the first half's columns. Each AllGather
# lands as a "comb" over global columns (8 strips of 512); the HOST
# permutes A's columns into [comb-A | comb-B] order before upload and
# un-permutes the output columns after download, so every device-side
# access stays contiguous. The CC stream has a fixed ~50us init that
# floors the first collective's completion near ~100us regardless of
# doorbell time, so everything except the final column-scale+store is
# scheduled before/under the collective windows.
#
# Engine assignment (driven by measured DVE fast-mode rules: 2x/4x
# modes exist only for copy/cast/tensor_scalar/tensor_tensor/
# tensor_reduce uops, and every STREAMED operand must be 2-byte,
# innermost step 1, >=2 elems, 4B-aligned; per-partition scalar APs are
# exempt and must in fact be f32):
#   row sums:  ACT-engine Copy-activations with f32 accum_out (7.1us
#              per [128,8192] tile, exact f32 degree) for the middle
#              tiles, chasing the loads; DVE grouped tensor_reduce
#              ([128,32,128] -> bf16 [128,32], then a tiny f32 second
#              stage) for tiles 0,1,7 so the first/last sums complete
#              with minimum latency. bf16 group partials cost ~1e-4
#              extra rel err (measured).
#   row scale: DVE tensor_scalar, 4x mode, f32 per-partition scalar
#              (1.28us per [128,4096] chunk) -- all 16 chunks run in
#              otherwise-idle DVE time BEFORE the collectives finish.
#   col scale: DVE tensor_mul vs the broadcast cvec, 2x mode (2.29us
#              per chunk), in place on the resident tiles -- the ONLY
#              compute left after the gathers.
#
# The gathered vector is replicated across partitions by broadcast-DMA;
# a single such DMA serializes ~one descriptor per partition, so each
# comb half is partition-sliced into 3 concurrent DMAs (43/43/42
# partitions) across the three HWDGE queues (~3x faster wall-clock).
#
# Queue discipline: HWDGE queues execute in order. Loads round-robin
# over Sync/Activation/GpSimd; collective doorbells are non-blocking on
# GpSimd (verified in trace) so its later loads proceed under CC#1; no
# collective-gated transfer is ever queued ahead of ungated work.

import numpy as np

N = 8192
NCORES = 8
R = N // NCORES   # 1024 rows per core
P = 128           # SBUF partitions
T = R // P        # 8 row-tiles of [128, 8192] per core
TH = T // 2       # row-tiles per collective half
HC = N // 2       # columns per comb half (4096)
LW = 4096         # load chunk width (1MB bf16)
GE = 128          # elements per reduce group
GT = N // GE      # groups per tile (64)
DVE_SUM_TILES = (0, 1, 7)  # tiles summed on DVE (grouped); rest on ACT

_cache = {}


def _perm():
    # device column order: [comb-A | comb-B];
    # comb-A = global cols c*1024 + [0,512), comb-B = c*1024 + [512,1024)
    idx = []
    for half in range(2):
        for c in range(NCORES):
            s = c * R + half * (R // 2)
            idx.extend(range(s, s + R // 2))
    return np.asarray(idx, dtype=np.int64)


def _build():
    import concourse.bacc as bacc
    import concourse.mybir as mybir
    import concourse.tile as tile
    from concourse import masks

    f32 = mybir.dt.float32
    bf16 = mybir.dt.bfloat16
    X = mybir.AxisListType.X
    mult = mybir.AluOpType.mult
    Copy = mybir.ActivationFunctionType.Copy

    nc = bacc.Bacc(
        "TRN2", target_bir_lowering=False, debug=False, num_devices=NCORES
    )
    a = nc.dram_tensor("a_shard", [R, N], bf16, kind="ExternalInput").ap()
    out = nc.dram_tensor("out_shard", [R, N], bf16, kind="ExternalOutput").ap()

    a_t = a.rearrange("(t p) n -> t p n", p=P)
    o_t = out.rearrange("(t p) n -> t p n", p=P)

    with tile.TileContext(nc) as tc:
        with (
            tc.tile_pool(name="cpool", bufs=1) as cpool,
            tc.tile_pool(name="vpool", bufs=1) as vpool,
            tc.tile_pool(name="psum", bufs=1, space="PSUM") as psum,
            tc.tile_pool(name="dram", bufs=1, space="DRAM") as dram,
        ):
            big = [
                cpool.tile([P, N], bf16, tag=f"c{t}", name=f"c{t}")
                for t in range(T)
            ]
            gsum = vpool.tile([P, len(DVE_SUM_TILES) * GT], bf16, tag="gsum")
            dsum = vpool.tile([P, T], f32, tag="dsum")
            dinv = vpool.tile([P, T], f32, tag="dinv")
            ident = vpool.tile([P, P], f32, tag="ident")
            cvec = vpool.tile([P, N], bf16, tag="cvec")
            dinv_tp = [
                vpool.tile([TH, P], bf16, tag=f"dtp{g}", name=f"dtp{g}")
                for g in range(2)
            ]
            dinv_tpp = [
                psum.tile([TH, P], f32, tag=f"tp{g}", name=f"tp{g}")
                for g in range(2)
            ]
            dloc = dram.tile([1, R], bf16, tag="dloc")
            dcomb = dram.tile([1, N], bf16, tag="dcomb")

            masks.make_identity(nc, ident[:, :])

            LQ = [nc.sync, nc.scalar, nc.gpsimd]
            nld = 0
            gslot = {t: i for i, t in enumerate(DVE_SUM_TILES)}

            def load_and_sum(t):
                nonlocal nld
                for h in range(N // LW):
                    cols = slice(h * LW, (h + 1) * LW)
                    LQ[nld % 3].dma_start(out=big[t][:, cols], in_=a_t[t][:, cols])
                    nld += 1
                    if t in gslot:
                        gs = slice(
                            gslot[t] * GT + h * (GT // 2),
                            gslot[t] * GT + (h + 1) * (GT // 2),
                        )
                        # bf16 group partials cost ~1e-4 extra rel err
                        # (final 64->1 stage below accumulates in f32)
                        # and buy the 2x DVE mode an f32 output forfeits
                        with nc.allow_low_precision(
                            reason="bf16 group partials, final sum f32"
                        ):
                            nc.vector.reduce_sum(
                                out=gsum[:, gs],
                                in_=big[t][:, cols].rearrange(
                                    "p (g e) -> p g e", e=GE
                                ),
                                axis=X,
                            )
                if t in gslot:
                    nc.vector.reduce_sum(
                        out=dsum[:, t : t + 1],
                        in_=gsum[:, gslot[t] * GT : (gslot[t] + 1) * GT],
                        axis=X,
                    )
                else:
                    # in-place Copy on ACT; the f32 accumulator output is
                    # the exact row sum, and the tile data is unchanged
                    nc.scalar.activation(
                        out=big[t][:, :],
                        in_=big[t][:, :],
                        func=Copy,
                        accum_out=dsum[:, t : t + 1],
                    )

            def gather_half(g):
                # d^-1/2 for row-tiles [g*TH, (g+1)*TH): sqrt+reciprocal
                # (ACT Rsqrt is banned for accuracy), PE-transpose so the
                # collective input is one contiguous row-ordered write.
                ts = slice(g * TH, (g + 1) * TH)
                nc.scalar.sqrt(dsum[:, ts], dsum[:, ts])
                nc.vector.reciprocal(dinv[:, ts], dsum[:, ts])
                nc.tensor.transpose(dinv_tpp[g][:, :], dinv[:, ts], ident[:, :])
                nc.scalar.copy(dinv_tp[g][:, :], dinv_tpp[g][:, :])
                rs = slice(g * (R // 2), (g + 1) * (R // 2))
                nc.gpsimd.dma_start(out=dloc[0, rs], in_=dinv_tp[g][:, :])
                nc.gpsimd.collective_compute(
                    "AllGather",
                    mybir.AluOpType.bypass,
                    replica_groups=[list(range(NCORES))],
                    ins=[dloc[0, rs].opt()],
                    outs=[dcomb[0, g * HC : (g + 1) * HC].opt()],
                )

            def rowscale(t, g):
                # DVE tensor_scalar hits the 4x mode (bf16 in/out, f32
                # per-partition scalar rides the exempt scalar port)
                cols = slice(g * HC, (g + 1) * HC)
                nc.vector.tensor_scalar(
                    out=big[t][:, cols],
                    in0=big[t][:, cols],
                    scalar1=dinv[:, t : t + 1],
                    scalar2=None,
                    op0=mult,
                )

            for t in range(TH):
                load_and_sum(t)
            gather_half(0)
            # all row-scaling for tiles 0-3 burns otherwise-idle DVE time
            # under the tile 4-7 loads and the collective windows
            for t in range(TH):
                rowscale(t, 0)
                rowscale(t, 1)
            for t in range(TH, T):
                load_and_sum(t)
            gather_half(1)
            for t in range(TH, T):
                rowscale(t, 0)
                rowscale(t, 1)

            # replicate the gathered halves across all 128 partitions;
            # ONE broadcast DMA serializes ~a descriptor per partition,
            # so slice each half across the three queues by partitions
            PS = [0, 43, 86, P]
            for g in range(2):
                cols = slice(g * HC, (g + 1) * HC)
                for q in range(3):
                    rows = slice(PS[q], PS[q + 1])
                    LQ[q].dma_start(
                        out=cvec[rows, cols],
                        in_=dcomb[0:1, cols].to_broadcast(
                            (PS[q + 1] - PS[q], HC)
                        ),
                    )
                for t in range(T):
                    nc.vector.tensor_mul(
                        big[t][:, cols], big[t][:, cols], cvec[:, cols]
                    )
                    LQ[t % 3].dma_start(out=o_t[t][:, cols], in_=big[t][:, cols])

    nc.compile()
    return nc


def kernel(adjacency_matrix, _trace=False):
    from concourse.bass_utils import run_bass_kernel_spmd
    import ml_dtypes

    A = np.asarray(adjacency_matrix)
    assert A.shape == (N, N), A.shape
    perm = _perm()
    Ab = np.ascontiguousarray(A.astype(ml_dtypes.bfloat16)[:, perm])

    if "nc" not in _cache:
        _cache["nc"] = _build()
    nc = _cache["nc"]

    in_maps = [{"a_shard": Ab[c * R : (c + 1) * R]} for c in range(NCORES)]
    res = run_bass_kernel_spmd(
        nc, in_maps, core_ids=list(range(NCORES)), trace=_trace
    )
    _cache["last"] = res
    dev = np.concatenate(
        [res.results[c]["out_shard"] for c in range(NCORES)], axis=0
    )
    full = np.empty((N, N), dtype=ml_dtypes.bfloat16)
    full[:, perm] = dev
    return full.astype(np.float32)


# revision 31
# speedup vs baseline: 1.1894x; 1.1894x over previous
# Laplacian normalization kernel for Trainium2 (8 NeuronCores, SPMD).
#
# out = d^-1/2[:, None] * A * d^-1/2[None, :],  d_i = sum_j A[i, j],  A: [8192, 8192] f32
#
# The rel-err gate is 2e-2; bf16 end-to-end measures ~1.35e-2 max rel
# err on this distribution (A, out, and the gathered scale vector in
# bf16; degree accumulation and d^-1/2 in f32), so the whole data path
# runs in bf16: HBM traffic per core is 32MB (16MB in + 16MB out) vs
# 88MB for the f32 two-pass version, and the full 16MB shard stays
# resident in SBUF (128KB/partition) so nothing is read twice.
#
# Sharding: row-wise across 8 cores (1024 rows each). Row sums are
# local; column scaling needs the full d^-1/2 [8192], gathered in TWO
# bf16 AllGathers (local rows 0-511, then 512-1023) so the second one
# overlaps the scale+store of the first half's columns. Each AllGather
# lands as a "comb" over global columns (8 strips of 512); the HOST
# permutes A's columns into [comb-A | comb-B] order before upload and
# un-permutes the output columns after download, so every device-side
# access stays contiguous. The CC stream has a fixed ~50us init that
# floors the first collective's completion near ~100us regardless of
# doorbell time, so everything except the final column-scale+store is
# scheduled before/under the collective windows.
#
# Engine assignment (driven by measured DVE fast-mode rules: 2x/4x
# modes exist only for copy/cast/tensor_scalar/tensor_tensor/
# tensor_reduce uops, and every STREAMED operand must be 2-byte,
# innermost step 1, >=2 elems, 4B-aligned; per-partition scalar APs are
# exempt and must in fact be f32):
#   row sums:  ACT-engine Copy-activations with f32 accum_out (7.1us
#              per [128,8192] tile, exact f32 degree) for the middle
#              tiles, chasing the loads; DVE grouped tensor_reduce
#              ([128,32,128] -> bf16 [128,32], then a tiny f32 second
#              stage) for tiles 0,1,7 so the first/last sums complete
#              with minimum latency. bf16 group partials cost ~1e-4
#              extra rel err (measured).
#   row scale: DVE tensor_scalar, 4x mode, f32 per-partition scalar
#              (1.28us per [128,4096] chunk) -- all 16 chunks run in
#              otherwise-idle DVE time BEFORE the collectives finish.
#   col scale: DVE tensor_mul vs the broadcast cvec, 2x mode (2.29us
#              per chunk), in place on the resident tiles -- the ONLY
#              compute left after the gathers.
#
# The gathered vector is replicated across partitions by broadcast-DMA;
# a single such DMA serializes ~one descriptor per partition, so each
# comb half is partition-sliced into 3 concurrent DMAs (43/43/42
# partitions) across the three HWDGE queues (~3x faster wall-clock).
#
# Queue discipline: HWDGE queues execute in order. Loads round-robin
# over Sync/Activation/GpSimd; collective doorbells are non-blocking on
# GpSimd (verified in trace) so its later loads proceed under CC#1; no
# collective-gated transfer is ever queued ahead of ungated work.

import numpy as np

N = 8192
NCORES = 8
R = N // NCORES   # 1024 rows per core
P = 128           # SBUF partitions
T = R // P        # 8 row-tiles of [128, 8192] per core
TH = T // 2       # row-tiles per collective half
HC = N // 2       # columns per comb half (4096)
LW = 4096         # load chunk width (1MB bf16)
GE = 128          # elements per reduce group
GT = N // GE      # groups per tile (64)
DVE_SUM_TILES = (0, 1, 7)  # tiles summed on DVE (grouped); rest on ACT

_cache = {}


def _perm():
    # device column order: [comb-A | comb-B];
    # comb-A = global cols c*1024 + [0,512), comb-B = c*1024 + [512,1024)
    idx = []
    for half in range(2):
        for c in range(NCORES):
            s = c * R + half * (R // 2)
            idx.extend(range(s, s + R // 2))
    return np.asarray(idx, dtype=np.int64)


def _build():
    import concourse.bacc as bacc
    import concourse.mybir as mybir
    import concourse.tile as tile
    from concourse import masks

    f32 = mybir.dt.float32
    bf16 = mybir.dt.bfloat16
    X = mybir.AxisListType.X
    mult = mybir.AluOpType.mult
    Copy = mybir.ActivationFunctionType.Copy

    nc = bacc.Bacc(
        "TRN2", target_bir_lowering=False, debug=False, num_devices=NCORES
    )
    a = nc.dram_tensor("a_shard", [R, N], bf16, kind="ExternalInput").ap()
    out = nc.dram_tensor("out_shard", [R, N], bf16, kind="ExternalOutput").ap()

    a_t = a.rearrange("(t p) n -> t p n", p=P)
    o_t = out.rearrange("(t p) n -> t p n", p=P)

    with tile.TileContext(nc) as tc:
        with (
            tc.tile_pool(name="cpool", bufs=1) as cpool,
            tc.tile_pool(name="vpool", bufs=1) as vpool,
            tc.tile_pool(name="psum", bufs=1, space="PSUM") as psum,
            tc.tile_pool(name="dram", bufs=1, space="DRAM") as dram,
        ):
            big = [
                cpool.tile([P, N], bf16, tag=f"c{t}", name=f"c{t}")
                for t in range(T)
            ]
            gsum = vpool.tile([P, len(DVE_SUM_TILES) * GT], bf16, tag="gsum")
            dsum = vpool.tile([P, T], f32, tag="dsum")
            dinv = vpool.tile([P, T], f32, tag="dinv")
            ident = vpool.tile([P, P], f32, tag="ident")
            cvec = vpool.tile([P, N], bf16, tag="cvec")
            dinv_tp = [
                vpool.tile([TH, P], bf16, tag=f"dtp{g}", name=f"dtp{g}")
                for g in range(2)
            ]
            dinv_tpp = [
                psum.tile([TH, P], f32, tag=f"tp{g}", name=f"tp{g}")
                for g in range(2)
            ]
            # separate tensors per collective half: a shared tensor makes
            # the dependency tracker serialize CC#2's input write behind
            # CC#1's read (WAR at tensor granularity), delaying CC#2 by a
            # full collective latency (measured +40us)
            dloc = [
                dram.tile([1, R // 2], bf16, tag=f"dl{g}", name=f"dl{g}")
                for g in range(2)
            ]
            dcomb = [
                dram.tile([1, HC], bf16, tag=f"dc{g}", name=f"dc{g}")
                for g in range(2)
            ]

            masks.make_identity(nc, ident[:, :])

            LQ = [nc.sync, nc.scalar, nc.gpsimd]
            nld = 0
            gslot = {t: i for i, t in enumerate(DVE_SUM_TILES)}

            def load_and_sum(t):
                nonlocal nld
                LQ[nld % 3].dma_start(out=big[t][:, :], in_=a_t[t][:, :])
                nld += 1
                if t in gslot:
                    gs = slice(gslot[t] * GT, (gslot[t] + 1) * GT)
                    # bf16 group partials cost ~1e-4 extra rel err
                    # (final 64->1 stage below accumulates in f32)
                    # and buy the 2x DVE mode an f32 output forfeits
                    with nc.allow_low_precision(
                        reason="bf16 group partials, final sum f32"
                    ):
                        nc.vector.reduce_sum(
                            out=gsum[:, gs],
                            in_=big[t][:, :].rearrange(
                                "p (g e) -> p g e", e=GE
                            ),
                            axis=X,
                        )
                    nc.vector.reduce_sum(
                        out=dsum[:, t : t + 1],
                        in_=gsum[:, gs],
                        axis=X,
                    )
                else:
                    # in-place Copy on ACT; the f32 accumulator output is
                    # the exact row sum, and the tile data is unchanged
                    nc.scalar.activation(
                        out=big[t][:, :],
                        in_=big[t][:, :],
                        func=Copy,
                        accum_out=dsum[:, t : t + 1],
                    )

            def chain_half(g):
                # d^-1/2 for row-tiles [g*TH, (g+1)*TH): sqrt+reciprocal
                # (ACT Rsqrt is banned for accuracy), PE-transpose so the
                # collective input is one contiguous row-ordered write.
                ts = slice(g * TH, (g + 1) * TH)
                nc.scalar.sqrt(dsum[:, ts], dsum[:, ts])
                nc.vector.reciprocal(dinv[:, ts], dsum[:, ts])
                nc.tensor.transpose(dinv_tpp[g][:, :], dinv[:, ts], ident[:, :])
                nc.scalar.copy(dinv_tp[g][:, :], dinv_tpp[g][:, :])

            def gather(g, q):
                nc.gpsimd.collective_compute(
                    "AllGather",
                    mybir.AluOpType.bypass,
                    replica_groups=[list(range(NCORES))],
                    ins=[dloc[g][0, :].opt()],
                    outs=[dcomb[g][0, :].opt()],
                )

            def rowscale(t, g):
                # DVE tensor_scalar hits the 4x mode (bf16 in/out, f32
                # per-partition scalar rides the exempt scalar port)
                cols = slice(g * HC, (g + 1) * HC)
                nc.vector.tensor_scalar(
                    out=big[t][:, cols],
                    in0=big[t][:, cols],
                    scalar1=dinv[:, t : t + 1],
                    scalar2=None,
                    op0=mult,
                )

            BW = HC // 2  # broadcast / scale / store chunk width (2048)

            def colscale_and_store(g, t, stq):
                # two 2048-wide tensor_mul chunks (finer chase of the
                # broadcast), one merged 1MB store per (tile, half)
                for b in range(2):
                    cols = slice(g * HC + b * BW, g * HC + (b + 1) * BW)
                    nc.vector.tensor_mul(
                        big[t][:, cols], big[t][:, cols], cvec[:, cols]
                    )
                half = slice(g * HC, (g + 1) * HC)
                stq.dma_start(out=o_t[t][:, half], in_=big[t][:, half])

            for t in range(TH):
                load_and_sum(t)
            chain_half(0)
            nc.gpsimd.dma_start(out=dloc[0][0, :], in_=dinv_tp[0][:, :])
            gather(0, nc.gpsimd)
            # all row-scaling for tiles 0-3 burns otherwise-idle DVE time
            # under the tile 4-7 loads and the collective windows
            for t in range(TH):
                rowscale(t, 0)
                rowscale(t, 1)
            for t in range(TH, T):
                load_and_sum(t)
            chain_half(1)

            # comb-A broadcast on Sync+Scalar (gated on CC#1). dloc_b's
            # write rides the Sync DMA ring BEHIND bcast-A chunk 1, and
            # CC#2's doorbell is data-gated on it -- so CC#2 cannot enter
            # the CC stream until the comb-A broadcast is through. A
            # regular DMA that overlaps an active collective window runs
            # at a ~10x-degraded trickle (measured), so this ordering
            # keeps the broadcast out of CC#2's window while CC#2 itself
            # overlaps the (unaffected) DVE scale compute and stores.
            for b in range(2):
                cols = slice(b * BW, (b + 1) * BW)
                [nc.sync, nc.scalar][b].dma_start(
                    out=cvec[:, cols],
                    in_=dcomb[0][0:1, cols].to_broadcast((P, BW)),
                )
            nc.sync.dma_start(out=dloc[1][0, :], in_=dinv_tp[1][:, :])
            gather(1, nc.gpsimd)
            for t in range(TH, T):
                rowscale(t, 0)
                rowscale(t, 1)

            # comb-A scale+store; stores on Sync+Scalar (GpSimd's ring is
            # parked behind CC#2's doorbell + bcast-B chunk 1)
            SA = [nc.sync, nc.scalar]
            for t in range(T):
                colscale_and_store(0, t, SA[t % 2])

            # comb-B broadcast on GpSimd+Scalar (gated on CC#2), then
            # scale+store on all three queues
            for b in range(2):
                src = slice(b * BW, (b + 1) * BW)
                [nc.gpsimd, nc.scalar][b].dma_start(
                    out=cvec[:, HC + b * BW : HC + (b + 1) * BW],
                    in_=dcomb[1][0:1, src].to_broadcast((P, BW)),
                )
            for t in range(T):
                colscale_and_store(1, t, LQ[t % 3])

    nc.compile()
    return nc


def kernel(adjacency_matrix, _trace=False):
    from concourse.bass_utils import run_bass_kernel_spmd
    import ml_dtypes

    A = np.asarray(adjacency_matrix)
    assert A.shape == (N, N), A.shape
    perm = _perm()
    Ab = np.ascontiguousarray(A.astype(ml_dtypes.bfloat16)[:, perm])

    if "nc" not in _cache:
        _cache["nc"] = _build()
    nc = _cache["nc"]

    in_maps = [{"a_shard": Ab[c * R : (c + 1) * R]} for c in range(NCORES)]
    res = run_bass_kernel_spmd(
        nc, in_maps, core_ids=list(range(NCORES)), trace=_trace
    )
    _cache["last"] = res
    dev = np.concatenate(
        [res.results[c]["out_shard"] for c in range(NCORES)], axis=0
    )
    full = np.empty((N, N), dtype=ml_dtypes.bfloat16)
    full[:, perm] = dev
    return full.astype(np.float32)
